# revision 1
# baseline (speedup 1.0000x reference)
"""MoE pointwise conv2d kernel for Trainium2 (8 NeuronCores, SPMD data-parallel).

Problem: out[b,o,h,w] = sum_i (sum_e routing[b,e] * weight[e,o,i]) * x[b,i,h,w]
Shapes:  x [64,384,28,28] f32, routing [64,8] f32, weight [8,384,384] f32.

Strategy (per core, 8 samples each):
  - Routing-combine (agg^T[b][i,o] = sum_e r[b,e] * w[e,o,i]) split across
    VectorE and GpSimdE via fused scalar_tensor_tensor MACs, written directly
    in matmul-lhsT layout (partition = i, free = (ki, o)).
  - Per-sample GEMM out[b] = agg_b @ x_b on TensorE, accumulating over 3
    k-tiles in PSUM (fp32).
  - ScalarE evacuates PSUM -> SBUF; HWDGE DMAs stream x in / out back.
  - Default fp16 wire format (x/weights/out on HBM + agg math) halves DMA
    volume and doubles DVE throughput; end-to-end rel err ~7e-4.
    KERNEL_F32=1 selects the fp32(+float32r matmul) build, rel err ~1.6e-4.
"""
import os
import sys

sys.path.insert(0, "/opt/trn_rl_repo")

import numpy as np
from contextlib import ExitStack

B, C_IN, C_OUT, E, H, W = 64, 384, 384, 8, 28, 28
HW = H * W            # 784
N_CORES = 8
BPC = B // N_CORES    # 8 samples per core
KI = C_IN // 128      # 3 k-tiles
MO = C_OUT // 128     # 3 output-partition tiles
NSPLIT = 2            # 784 -> 2 x 392 (<= 512 psum bank limit)
NCH = HW // NSPLIT    # 392
WCOL = KI * C_OUT     # 1152

USE_F16 = os.environ.get("KERNEL_F32", "0") != "1"

_cache = {}


def _build(use_f16=USE_F16, spl=WCOL, reps=1, serialize_reps=False, pair=True, agg_bufs=2, micro=True, quad=False, dense_rw=False, slack=True, slack2=False):
    import concourse.tile as tile
    import concourse.mybir as mybir
    from concourse import bacc
    from concourse.tile import add_dep_helper

    f32 = mybir.dt.float32
    f32r = mybir.dt.float32r
    f16 = mybir.dt.float16
    mult = mybir.AluOpType.mult
    add = mybir.AluOpType.add

    dio = f16 if use_f16 else f32        # wire dtype for wt/x/out
    dmm = f16 if use_f16 else f32r       # matmul operand dtype

    nc = bacc.Bacc("TRN2", target_bir_lowering=False, debug=False)
    x_d = nc.dram_tensor("x", [BPC, C_IN, HW], dio, kind="ExternalInput")
    rw_d = nc.dram_tensor("rw", [128 if dense_rw else 1, BPC * E], f32,
                          kind="ExternalInput")
    wt_d = nc.dram_tensor("wt", [E, 128, WCOL], dio, kind="ExternalInput")
    out_d = nc.dram_tensor("out", [reps * BPC, C_OUT, HW], dio,
                           kind="ExternalOutput")

    with tile.TileContext(nc) as tc:
        with ExitStack() as ctx:
            wt_pool = ctx.enter_context(tc.tile_pool(name="wt", bufs=E))
            rw_pool = ctx.enter_context(tc.tile_pool(name="rw", bufs=2))
            agg_pool = ctx.enter_context(tc.tile_pool(name="agg", bufs=max(agg_bufs, 4 if quad else (3 if slack2 else 2))))
            x_pool = ctx.enter_context(tc.tile_pool(name="xp", bufs=4 if slack2 else (3 if slack else 2)))
            out_pool = ctx.enter_context(tc.tile_pool(name="op", bufs=10 if slack2 else (8 if slack else 6)))
            ps_pool = ctx.enter_context(tc.tile_pool(name="ps", bufs=8 if slack2 else (6 if slack else 4), space="PSUM"))

            prev_out_dmas, cur_out_dmas = [], []
            pair_tiles = {}

            def _fence(inst):
                if serialize_reps:
                    for d in prev_out_dmas:
                        add_dep_helper(inst.ins, d.ins, reason="serialize reps")
                return inst

            for rep, b in ((r, b) for r in range(reps) for b in range(BPC)):
                if b == 0:
                    prev_out_dmas, cur_out_dmas = cur_out_dmas, []
                    rw_sb = rw_pool.tile([128, BPC * E], f32)
                    _fence(nc.sync.dma_start(
                        rw_sb[:],
                        rw_d[:] if dense_rw
                        else rw_d[:].to_broadcast((128, BPC * E))))
                    wt_sb, wt_dmas = [], []
                    for e in range(E):
                        t = wt_pool.tile([128, WCOL], dio)
                        wt_dmas.append(_fence(nc.sync.dma_start(t[:], wt_d[e])))
                        wt_sb.append(t)
                # ---- routing combine ----
                # DVE does cols [0:spl) with fused scalar_tensor_tensor MACs
                # (2-byte operands keep the 2x_1p DVE mode). GPSIMD cannot run
                # TensorScalarPtr (walrus rejects Pool), and its tensor_tensor
                # 2-op MAC measured ~33us/invocation WORSE on HW (shared-port
                # lock vs DVE packed modes) — keep spl == WCOL (DVE-only).
                # fp16 accumulator keeps every operand 2-byte -> 2x DVE mode
                GSZ = 4 if quad else 2
                if pair and b % GSZ == 0:
                    # emit the MAC chains of samples (b, b+1) interleaved so
                    # DVE hides each chain's op-to-op dependency latency
                    pr = []
                    for bb in range(b, b + GSZ):
                        a_ = agg_pool.tile([128, WCOL], f16 if use_f16 else f32,
                                           tag="aggT")
                        ar_ = agg_pool.tile([128, WCOL], dmm, tag="aggr")
                        pr.append((bb, a_, ar_))
                    for gi in range(GSZ):
                        pair_tiles[b + gi] = pr[gi][1:]
                    for e in range(E):
                        for bb, a_, ar_ in pr:
                            s = rw_sb[:, bb * E + e:bb * E + e + 1]
                            if e == 0:
                                nc.vector.tensor_scalar(
                                    a_[:], wt_sb[0][:], s, None, mult)
                            elif e < E - 1:
                                nc.vector.scalar_tensor_tensor(
                                    a_[:], wt_sb[e][:], s, a_[:], mult, add)
                            elif micro and b == BPC - GSZ:
                                for k3 in range(KI):
                                    cs = slice(k3 * C_OUT, (k3 + 1) * C_OUT)
                                    nc.vector.scalar_tensor_tensor(
                                        ar_[:, cs], wt_sb[e][:, cs], s,
                                        a_[:, cs], mult, add)
                            else:
                                nc.vector.scalar_tensor_tensor(
                                    ar_[:], wt_sb[e][:], s, a_[:], mult, add)
                if pair:
                    aggT, aggT_r = pair_tiles[b]
                    sc = lambda e: rw_sb[:, b * E + e:b * E + e + 1]
                else:
                    aggT = agg_pool.tile([128, WCOL], f16 if use_f16 else f32)
                    aggT_r = agg_pool.tile([128, WCOL], dmm, tag="aggr")
                    sc = lambda e: rw_sb[:, b * E + e:b * E + e + 1]
                if not pair:
                    nc.vector.tensor_scalar(
                        aggT[:, 0:spl], wt_sb[0][:, 0:spl], sc(0), None, mult
                    )
                    for e in range(1, E - 1):
                        nc.vector.scalar_tensor_tensor(
                            aggT[:, 0:spl], wt_sb[e][:, 0:spl], sc(e),
                            aggT[:, 0:spl], mult, add,
                        )
                    nc.vector.scalar_tensor_tensor(
                        aggT_r[:, 0:spl], wt_sb[E - 1][:, 0:spl], sc(E - 1),
                        aggT[:, 0:spl], mult, add,
                    )
                if spl < WCOL:
                    gw = WCOL - spl
                    gtmp = agg_pool.tile([128, gw], f16 if use_f16 else f32,
                                         tag="gtmp")
                    scb = lambda e: sc(e).to_broadcast((128, gw))
                    nc.gpsimd.tensor_tensor(
                        aggT[:, spl:], wt_sb[0][:, spl:], scb(0), mult)
                    for e in range(1, E - 1):
                        nc.gpsimd.tensor_tensor(
                            gtmp[:], wt_sb[e][:, spl:], scb(e), mult)
                        nc.gpsimd.tensor_tensor(
                            aggT[:, spl:], aggT[:, spl:], gtmp[:], add)
                    nc.gpsimd.tensor_tensor(
                        gtmp[:], wt_sb[E - 1][:, spl:], scb(E - 1), mult)
                    nc.gpsimd.tensor_tensor(
                        aggT_r[:, spl:], aggT[:, spl:], gtmp[:], add)

                # ---- load x_b ----
                x_sb = x_pool.tile([128, KI * HW], dmm)
                for ki in range(KI):
                    src = x_d[b, ki * 128:(ki + 1) * 128, :]
                    xi = _fence(nc.sync.dma_start(x_sb[:, ki * HW:(ki + 1) * HW],
                                                  src if use_f16 else src.bitcast(f32r)))
                    if micro and b < 2:
                        for wd in wt_dmas:
                            add_dep_helper(xi.ins, wd.ins,
                                           reason="x after wt (head trim)")

                # ---- per-sample GEMM ----
                for mo in range(MO):
                    for n in range(NSPLIT):
                        ps = ps_pool.tile([128, NCH], f32)
                        for ki in range(KI):
                            lhs = aggT_r[:, ki * C_OUT + mo * 128:
                                         ki * C_OUT + (mo + 1) * 128]
                            rhs = x_sb[:, ki * HW + n * NCH:
                                       ki * HW + (n + 1) * NCH]
                            nc.tensor.matmul(
                                ps[:], lhs, rhs,
                                start=(ki == 0), stop=(ki == KI - 1),
                            )
                        o_sb = out_pool.tile([128, NCH], dio)
                        nc.scalar.copy(o_sb[:], ps[:])
                        cur_out_dmas.append(nc.sync.dma_start(
                            out_d[rep * BPC + b, mo * 128:(mo + 1) * 128,
                                  n * NCH:(n + 1) * NCH],
                            o_sb[:],
                        ))
    nc.compile()
    return nc


def kernel(x: np.ndarray, routing_weights: np.ndarray, weight: np.ndarray,
           _trace: bool = False):
    from concourse.bass_utils import run_bass_kernel_spmd

    x = np.asarray(x, dtype=np.float32)
    routing_weights = np.ascontiguousarray(np.asarray(routing_weights, dtype=np.float32))
    weight = np.asarray(weight, dtype=np.float32)

    if "nc" not in _cache:
        _cache["nc"] = _build()
    nc = _cache["nc"]

    np_io = np.float16 if USE_F16 else np.float32

    # wt[e, p, ki*384 + o] = weight[e, o, ki*128 + p]
    wt = np.ascontiguousarray(
        weight.reshape(E, C_OUT, KI, 128).transpose(0, 3, 2, 1)
        .reshape(E, 128, WCOL).astype(np_io)
    )
    x_r = np.ascontiguousarray(x.reshape(B, C_IN, HW).astype(np_io))

    in_maps = []
    for c in range(N_CORES):
        sl = slice(c * BPC, (c + 1) * BPC)
        in_maps.append({
            "x": x_r[sl],
            "rw": np.ascontiguousarray(routing_weights[sl].reshape(1, BPC * E)),
            "wt": wt,
        })

    res = run_bass_kernel_spmd(nc, in_maps, core_ids=list(range(N_CORES)),
                               trace=_trace)
    out = np.concatenate([res.results[c]["out"] for c in range(N_CORES)], axis=0)
    if _trace:
        _cache["last_result"] = res
    return out.reshape(B, C_OUT, H, W).astype(np.float32)


if __name__ == "__main__":
    rng = np.random.default_rng(0)
    x = rng.standard_normal((B, C_IN, H, W), dtype=np.float32)
    rw = rng.random((B, E), dtype=np.float32)
    w = rng.standard_normal((E, C_OUT, C_IN), dtype=np.float32)
    got = kernel(x, rw, w)
    agg = np.einsum('be,eoi->boi', rw, w)
    want = np.einsum('boi,bihw->bohw', agg, x.reshape(B, C_IN, H, W))
    err = np.abs(got - want).max() / np.abs(want).max()
    print("rel err:", err)



# revision 2
# speedup vs baseline: 1.0653x; 1.0653x over previous
"""MoE pointwise conv2d kernel for Trainium2 (8 NeuronCores, SPMD data-parallel).

Problem: out[b,o,h,w] = sum_i (sum_e routing[b,e] * weight[e,o,i]) * x[b,i,h,w]
Shapes:  x [64,384,28,28] f32, routing [64,8] f32, weight [8,384,384] f32.

Strategy (per core, 8 samples each), fp16 wire format end-to-end:
  - Routing-combine agg^T[b][i, (ki,o)] = sum_e r[b,e] w[e]:
    ScalarE seeds each sample's accumulator with the expert-0 scaled copy
    (activation Copy with per-partition scale), then VectorE chains the
    remaining 7 experts as fused scalar_tensor_tensor MACs (fp16 operands
    keep the 2x_1p DVE perf mode).  Chains of a sample pair are interleaved
    to hide op-to-op latency; the final pair is emitted in ki-column chunks
    so TensorE can start on its k-tiles early (tail trim).
  - Per-sample GEMM out[b] = agg_b @ x_b on TensorE: psum tiles [128,784]
    spanning 2 banks, two accumulation groups (FD 512 in bank0, FD 272 in
    bank1), 3 k-tile accumulation each.
  - ScalarE evacuates each [128,784] psum tile in one op into a per-sample
    [128, 3*784] out tile.
  - One DMA per sample for x in (dram viewed [KI,128,HW], transposed AP) and
    out back (dram viewed [MO,128,HW]); 8 expert-weight DMAs + 1 routing DMA.
    25 DMAs/rep total (vs 81 in the 2-DMA-per-tile layout) to keep the
    ~0.6us/DMA HWDGE+SP dispatch cost off the critical path.
"""
import os
import sys

sys.path.insert(0, "/opt/trn_rl_repo")

import numpy as np
from contextlib import ExitStack

B, C_IN, C_OUT, E, H, W = 64, 384, 384, 8, 28, 28
HW = H * W            # 784
N_CORES = 8
BPC = B // N_CORES    # 8 samples per core
KI = C_IN // 128      # 3 k-tiles
MO = C_OUT // 128     # 3 output-partition tiles
WCOL = KI * C_OUT     # 1152
NSPLITS = ((0, 512), (512, 272))  # psum accumulation groups (bank-aligned)

_cache = {}


def _build(reps=1, serialize_reps=False, gsz=2, act_e0=True, tail_chunks=True):
    import concourse.tile as tile
    import concourse.mybir as mybir
    from concourse import bacc
    from concourse.tile import add_dep_helper

    f32 = mybir.dt.float32
    f16 = mybir.dt.float16
    mult = mybir.AluOpType.mult
    add = mybir.AluOpType.add

    nc = bacc.Bacc("TRN2", target_bir_lowering=False, debug=False)
    x_d = nc.dram_tensor("x", [BPC, KI, 128, HW], f16, kind="ExternalInput")
    rw_d = nc.dram_tensor("rw", [1, BPC * E], f32, kind="ExternalInput")
    wt_d = nc.dram_tensor("wt", [E, 128, WCOL], f16, kind="ExternalInput")
    out_d = nc.dram_tensor("out", [reps * BPC, MO, 128, HW], f16,
                           kind="ExternalOutput")

    with tile.TileContext(nc) as tc:
        with ExitStack() as ctx:
            wt_pool = ctx.enter_context(tc.tile_pool(name="wt", bufs=E))
            rw_pool = ctx.enter_context(tc.tile_pool(name="rw", bufs=2))
            agg_pool = ctx.enter_context(tc.tile_pool(name="agg", bufs=BPC))
            x_pool = ctx.enter_context(tc.tile_pool(name="xp", bufs=3))
            out_pool = ctx.enter_context(tc.tile_pool(name="op", bufs=3))
            ps_pool = ctx.enter_context(tc.tile_pool(name="ps", bufs=4,
                                                     space="PSUM"))

            prev_out_dmas, cur_out_dmas = [], []

            def _fence(inst):
                if serialize_reps:
                    for d in prev_out_dmas:
                        add_dep_helper(inst.ins, d.ins, reason="serialize reps")
                return inst

            for rep in range(reps):
                prev_out_dmas, cur_out_dmas = cur_out_dmas, []
                rw_sb = rw_pool.tile([128, BPC * E], f32)
                _fence(nc.sync.dma_start(
                    rw_sb[:], rw_d[:].to_broadcast((128, BPC * E))))
                wt_sb, wt_dmas = [], []
                for e in range(E):
                    t = wt_pool.tile([128, WCOL], f16)
                    wt_dmas.append(_fence(nc.sync.dma_start(t[:], wt_d[e])))
                    wt_sb.append(t)

                sc = lambda b, e: rw_sb[:, b * E + e:b * E + e + 1]

                # ---- expert-0 seed on ScalarE, all samples upfront ----
                agg = []
                for b in range(BPC):
                    a_ = agg_pool.tile([128, WCOL], f16, tag="aggT")
                    agg.append(a_)
                    if act_e0:
                        nc.scalar.mul(a_[:], wt_sb[0][:], sc(b, 0))
                # ---- experts 1..7 MAC chains on VectorE, pair-interleaved ----
                for g in range(0, BPC, gsz):
                    grp = range(g, g + gsz)
                    if not act_e0:
                        for bb in grp:
                            nc.vector.tensor_scalar(
                                agg[bb][:], wt_sb[0][:], sc(bb, 0), None, mult)
                    last = tail_chunks and g == BPC - gsz
                    for e in range(1, E):
                        if last and e == E - 1:
                            for k3 in range(KI):
                                cs = slice(k3 * C_OUT, (k3 + 1) * C_OUT)
                                for bb in grp:
                                    nc.vector.scalar_tensor_tensor(
                                        agg[bb][:, cs], wt_sb[e][:, cs],
                                        sc(bb, e), agg[bb][:, cs], mult, add)
                        else:
                            for bb in grp:
                                nc.vector.scalar_tensor_tensor(
                                    agg[bb][:], wt_sb[e][:], sc(bb, e),
                                    agg[bb][:], mult, add)

                # ---- per-sample GEMM + evac + out DMA ----
                for b in range(BPC):
                    x_sb = x_pool.tile([128, KI, HW], f16)
                    xi = _fence(nc.sync.dma_start(
                        x_sb[:], x_d[b].transpose([1, 0, 2])))
                    if b < 2:
                        for wd in wt_dmas:
                            add_dep_helper(xi.ins, wd.ins,
                                           reason="x after wt (head trim)")
                    o_sb = out_pool.tile([128, MO, HW], f16)
                    for mo in range(MO):
                        ps = ps_pool.tile([128, HW], f32)
                        for n0, nw in NSPLITS:
                            for ki in range(KI):
                                lhs = agg[b][:, ki * C_OUT + mo * 128:
                                             ki * C_OUT + (mo + 1) * 128]
                                nc.tensor.matmul(
                                    ps[:, n0:n0 + nw],
                                    lhs, x_sb[:, ki, n0:n0 + nw],
                                    start=(ki == 0), stop=(ki == KI - 1),
                                )
                        nc.scalar.copy(o_sb[:, mo, :], ps[:])
                    cur_out_dmas.append(nc.sync.dma_start(
                        out_d[rep * BPC + b].transpose([1, 0, 2]), o_sb[:]))
    nc.compile()
    return nc


def kernel(x: np.ndarray, routing_weights: np.ndarray, weight: np.ndarray,
           _trace: bool = False):
    from concourse.bass_utils import run_bass_kernel_spmd

    x = np.asarray(x, dtype=np.float32)
    routing_weights = np.ascontiguousarray(
        np.asarray(routing_weights, dtype=np.float32))
    weight = np.asarray(weight, dtype=np.float32)

    if "nc" not in _cache:
        _cache["nc"] = _build()
    nc = _cache["nc"]

    # wt[e, p, ki*384 + o] = weight[e, o, ki*128 + p]
    wt = np.ascontiguousarray(
        weight.reshape(E, C_OUT, KI, 128).transpose(0, 3, 2, 1)
        .reshape(E, 128, WCOL).astype(np.float16)
    )
    x_r = np.ascontiguousarray(
        x.reshape(B, KI, 128, HW).astype(np.float16))

    in_maps = []
    for c in range(N_CORES):
        sl = slice(c * BPC, (c + 1) * BPC)
        in_maps.append({
            "x": x_r[sl],
            "rw": np.ascontiguousarray(
                routing_weights[sl].reshape(1, BPC * E)),
            "wt": wt,
        })

    res = run_bass_kernel_spmd(nc, in_maps, core_ids=list(range(N_CORES)),
                               trace=_trace)
    out = np.concatenate([res.results[c]["out"] for c in range(N_CORES)],
                         axis=0)
    if _trace:
        _cache["last_result"] = res
    return out.reshape(B, C_OUT, H, W).astype(np.float32)


if __name__ == "__main__":
    rng = np.random.default_rng(0)
    x = rng.standard_normal((B, C_IN, H, W), dtype=np.float32)
    rw = rng.random((B, E), dtype=np.float32)
    w = rng.standard_normal((E, C_OUT, C_IN), dtype=np.float32)
    got = kernel(x, rw, w)
    agg = np.einsum('be,eoi->boi', rw, w)
    want = np.einsum('boi,bihw->bohw', agg, x.reshape(B, C_IN, H, W))
    err = np.abs(got - want).max() / np.abs(want).max()
    print("rel err:", err)


# revision 11
# speedup vs baseline: 1.5722x; 1.4758x over previous
"""MoE pointwise conv2d kernel for Trainium2 (8 NeuronCores, SPMD data-parallel).

Problem: out[b,o,h,w] = sum_i (sum_e routing[b,e] * weight[e,o,i]) * x[b,i,h,w]
Shapes:  x [64,384,28,28] f32, routing [64,8] f32, weight [8,384,384] f32.

Strategy (per core, 8 samples each), fp16 wire format end-to-end:
  - Routing-combine runs on TensorE (measured DVE scalar_tensor_tensor runs
    1x-mode only => a DVE MAC chain costs ~71us/core; TensorE does the same
    contraction in ~8us):
      The host expands routing into a sparse matrix
        rq[(e,o16), (b,o16')] = r[b,e] * delta(o16,o16')   [128 x 128]
      and pre-permutes weights to
        wt[(e,o16), (ki, chunk, i_lo)]                     [128 x 9216]
      so one matmul per (ki, o-chunk of 16) computes
        agg^T[i_lo, (b, o16)] = sum_e r[b,e] w[e, chunk*16+o16, ki*128+i_lo]
      for ALL 8 samples at once: 72 matmuls, FD=128, fp32 PSUM accumulate.
  - ScalarE evacuates agg psum tiles ([128,512], 4 chunks each) into a
    [128, 9216] f16 staging tile laid out (ki, chunk, b, o16).
  - Main GEMM out[b] = agg_b @ x_b on TensorE: lhsT tiles are strided 3D APs
    into staging (8 chunks x 16 cols per (ki,mo,b)); psum [128,784] spanning
    2 banks, accumulation groups FD 512 + 272 over 3 k-tiles.
  - PSUM out evacuation alternates ScalarE/VectorE; one [128, 3*784] out
    tile per sample.
  - DMAs per rep: 3 wt (split by ki) + 1 rq + 8 x + 8 out = 20.
"""
import os
import sys

sys.path.insert(0, "/opt/trn_rl_repo")

import numpy as np
from contextlib import ExitStack

B, C_IN, C_OUT, E, H, W = 64, 384, 384, 8, 28, 28
HW = H * W            # 784
N_CORES = 8
BPC = B // N_CORES    # 8 samples per core
KI = C_IN // 128      # 3 k-tiles
MO = C_OUT // 128     # 3 output-partition tiles
OC = 16               # o-values per chunk
NCH = C_OUT // OC     # 24 o-chunks
CPK = NCH * 128       # staging cols per ki (3072)
SCOL = KI * CPK       # staging cols total (9216)
NSPLITS = ((0, 512), (512, 272))  # psum accumulation groups (bank-aligned)

_cache = {}


def _build(reps=1, serialize_reps=False, small_out=False, cg4=4,
           evac_split=True):
    import concourse.tile as tile
    import concourse.mybir as mybir
    from concourse import bacc
    from concourse.tile import add_dep_helper

    f32 = mybir.dt.float32
    f16 = mybir.dt.float16

    nc = bacc.Bacc("TRN2", target_bir_lowering=False, debug=False)
    x_d = nc.dram_tensor("x", [BPC, KI, 128, HW], f16, kind="ExternalInput")
    rq_d = nc.dram_tensor("rq", [128, 128], f16, kind="ExternalInput")
    wt_d = nc.dram_tensor("wt", [KI, 128, CPK], f16, kind="ExternalInput")
    out_d = nc.dram_tensor("out", [(1 if small_out else reps) * BPC, MO, 128, HW],
                           f16, kind="ExternalOutput")

    with tile.TileContext(nc) as tc:
        with ExitStack() as ctx:
            wt_pool = ctx.enter_context(tc.tile_pool(name="wt", bufs=2))
            rq_pool = ctx.enter_context(tc.tile_pool(name="rq", bufs=2))
            stag_pool = ctx.enter_context(tc.tile_pool(name="st", bufs=2))
            x_pool = ctx.enter_context(tc.tile_pool(name="xp", bufs=3))
            out_pool = ctx.enter_context(tc.tile_pool(name="op", bufs=3))
            psa_pool = ctx.enter_context(tc.tile_pool(name="pa", bufs=2,
                                                      space="PSUM"))
            psm_pool = ctx.enter_context(tc.tile_pool(name="pm", bufs=3,
                                                      space="PSUM"))

            prev_out_dmas, cur_out_dmas = [], []

            def _fence(inst):
                if serialize_reps:
                    for d in prev_out_dmas:
                        add_dep_helper(inst.ins, d.ins, reason="serialize reps")
                return inst

            for rep in range(reps):
                prev_out_dmas, cur_out_dmas = cur_out_dmas, []
                rq_sb = rq_pool.tile([128, 128], f16)
                _fence(nc.sync.dma_start(rq_sb[:], rq_d[:]))
                wt_sb = wt_pool.tile([128, SCOL], f16)
                wt_dmas = []
                for ki in range(KI):
                    wt_dmas.append(_fence(nc.sync.dma_start(
                        wt_sb[:, ki * CPK:(ki + 1) * CPK], wt_d[ki])))

                # ---- routing-combine on TensorE ----
                # stag[(ki, chunk, o16, b)] = agg[b, chunk*16+o16, ki*128+p]
                stag = stag_pool.tile([128, SCOL], f16)
                for ki in range(KI):
                    for cg in range(NCH // cg4):
                        ps = psa_pool.tile([128, cg4 * 128], f32)
                        for c4 in range(cg4):
                            chunk = cg * cg4 + c4
                            nc.tensor.matmul(
                                ps[:, c4 * 128:(c4 + 1) * 128],
                                wt_sb[:, (ki * NCH + chunk) * 128:
                                      (ki * NCH + chunk + 1) * 128],
                                rq_sb[:],
                                start=True, stop=True,
                            )
                        base = (ki * NCH + cg * cg4) * 128
                        nc.scalar.copy(stag[:, base:base + cg4 * 128], ps[:])

                # ---- per-sample GEMM + evac + out DMA ----
                for b in range(BPC):
                    x_sb = x_pool.tile([128, KI, HW], f16)
                    xi = _fence(nc.sync.dma_start(
                        x_sb[:], x_d[b].transpose([1, 0, 2])))
                    if b < 2:
                        for wd in wt_dmas:
                            add_dep_helper(xi.ins, wd.ins,
                                           reason="x after wt (head trim)")
                    o_sb = out_pool.tile([128, MO, HW], f16)
                    for mo in range(MO):
                        ps = psm_pool.tile([128, HW], f32)
                        for n0, nw in NSPLITS:
                            for ki in range(KI):
                                base = (ki * NCH + mo * (NCH // MO)) * 128
                                lhs = stag[:, base + b:base + 1024:BPC]
                                nc.tensor.matmul(
                                    ps[:, n0:n0 + nw],
                                    lhs, x_sb[:, ki, n0:n0 + nw],
                                    start=(ki == 0), stop=(ki == KI - 1),
                                )
                        if evac_split and mo >= 1:
                            nc.vector.tensor_copy(o_sb[:, mo, :], ps[:])
                        else:
                            nc.scalar.copy(o_sb[:, mo, :], ps[:])
                    cur_out_dmas.append(nc.sync.dma_start(
                        out_d[(0 if small_out else rep) * BPC + b]
                        .transpose([1, 0, 2]), o_sb[:]))
    nc.compile()
    return nc


def _host_prep(x, routing_weights, weight):
    """Full inputs -> per-core in_maps with the kernel's dram layouts."""
    # wt[ki][e*16+o16, chunk*128 + i_lo] = weight[e, chunk*16+o16, ki*128+i_lo]
    wt = np.ascontiguousarray(
        weight.reshape(E, NCH, OC, KI, 128)      # e, chunk, o16, ki, i_lo
        .transpose(3, 0, 2, 1, 4)                # ki, e, o16, chunk, i_lo
        .reshape(KI, 128, CPK).astype(np.float16))
    x_r = np.ascontiguousarray(x.reshape(B, KI, 128, HW).astype(np.float16))

    in_maps = []
    for c in range(N_CORES):
        r_core = routing_weights[c * BPC:(c + 1) * BPC]   # [BPC, E]
        rq = np.zeros((E, OC, OC, BPC), dtype=np.float16)
        for o16 in range(OC):
            rq[:, o16, o16, :] = r_core.T.astype(np.float16)
        in_maps.append({
            "x": x_r[c * BPC:(c + 1) * BPC],
            "rq": np.ascontiguousarray(rq.reshape(128, 128)),
            "wt": wt,
        })
    return in_maps


def kernel(x: np.ndarray, routing_weights: np.ndarray, weight: np.ndarray,
           _trace: bool = False):
    from concourse.bass_utils import run_bass_kernel_spmd

    x = np.asarray(x, dtype=np.float32)
    routing_weights = np.ascontiguousarray(
        np.asarray(routing_weights, dtype=np.float32))
    weight = np.asarray(weight, dtype=np.float32)

    if "nc" not in _cache:
        _cache["nc"] = _build()
    nc = _cache["nc"]

    in_maps = _host_prep(x, routing_weights, weight)
    res = run_bass_kernel_spmd(nc, in_maps, core_ids=list(range(N_CORES)),
                               trace=_trace)
    out = np.concatenate([res.results[c]["out"] for c in range(N_CORES)],
                         axis=0)
    if _trace:
        _cache["last_result"] = res
    return out.reshape(B, C_OUT, H, W).astype(np.float32)


if __name__ == "__main__":
    rng = np.random.default_rng(0)
    x = rng.standard_normal((B, C_IN, H, W), dtype=np.float32)
    rw = rng.random((B, E), dtype=np.float32)
    w = rng.standard_normal((E, C_OUT, C_IN), dtype=np.float32)
    got = kernel(x, rw, w)
    agg = np.einsum('be,eoi->boi', rw, w)
    want = np.einsum('boi,bihw->bohw', agg, x.reshape(B, C_IN, H, W))
    err = np.abs(got - want).max() / np.abs(want).max()
    print("rel err:", err)
